# revision 28
# baseline (speedup 1.0000x reference)
"""Domain-specific BatchNorm (nn_DSBatchNorm) Trainium2 Bass kernel.

Data-parallel over rows across 8 NeuronCores. Per core:
  pass A: segmented per-domain sums/sumsq/counts via bf16 one-hot matmuls
          into PSUM (one-hot is exact in bf16; x/x^2 cast to bf16 only
          perturbs the stats by ~1e-5 relative, well inside tolerance)
  tiny AllReduce of the [8, 2F+1] packed stats
  table math: A = gamma*inv*nz, B = beta*nz - A*mean_e  (per-domain [8,F])
  pass B: per row-tile, gather per-row coeffs A_rows/B_rows with a single
          bf16 matmul each, using a hi/lo bf16 split of the f32 tables
          stacked along K (A ~= A_hi + A_lo reconstructed exactly in the
          f32 PSUM accumulator), then out = x*A_rows + B_rows on DVE.
"""

import sys

if "/opt/trn_rl_repo" not in sys.path:
    sys.path.insert(0, "/opt/trn_rl_repo")

import numpy as np

import concourse.bacc as bacc
import concourse.bass as bass
import concourse.tile as tile
from concourse import mybir
from concourse.bass_utils import run_bass_kernel_spmd

N_CORES = 8
N, F, D = 262144, 512, 8
NS = N // N_CORES  # rows per core
P = 128
T = NS // P  # row-tiles per core
CHUNK = 8  # row-tiles per DMA chunk (2 MB)
NCHUNKS = T // CHUNK
EPS = 1e-5
f32 = mybir.dt.float32
bf16 = mybir.dt.bfloat16
i32 = mybir.dt.int32

_CACHE = {}

# test.py can flip this to get a traced run; grading path leaves it False
TRACE = False
LAST_RESULTS = None


def _build():
    AluOp = mybir.AluOpType
    nc = bacc.Bacc(
        "TRN2", target_bir_lowering=False, debug=False, num_devices=N_CORES
    )

    x = nc.dram_tensor("x", [NS, F], f32, kind="ExternalInput")
    yf = nc.dram_tensor("yf", [NS], f32, kind="ExternalInput")
    gamma = nc.dram_tensor("gamma", [D, F], f32, kind="ExternalInput")
    beta = nc.dram_tensor("beta", [D, F], f32, kind="ExternalInput")
    out = nc.dram_tensor("out", [NS, F], f32, kind="ExternalOutput")

    ident_c = nc.inline_tensor(np.eye(P, dtype=np.float32), name="ident_c")

    # p-major row mapping: partition p, tile t <-> row p*T + t. Stats are
    # permutation-invariant and load/store/one-hot all use the same mapping,
    # so this is just a DMA-friendly tiling (16 KB contiguous per partition
    # per chunk).
    x_r = x[:].rearrange("(p t) f -> p t f", t=T)
    out_r = out[:].rearrange("(p t) f -> p t f", t=T)
    y_r = yf[:].rearrange("(p t) -> p t", t=T)

    with tile.TileContext(nc) as tc:
        with (
            tc.tile_pool(name="consts", bufs=1) as consts,
            tc.tile_pool(name="tables", bufs=1) as tables,
            tc.tile_pool(name="xc", bufs=3) as xcp,
            tc.tile_pool(name="xb", bufs=4) as xbp,
            tc.tile_pool(name="xsq", bufs=4) as xsqp,
            tc.tile_pool(name="oh", bufs=4) as ohp,
            tc.tile_pool(name="oc", bufs=3) as ocp,
            tc.tile_pool(name="oh2", bufs=2) as oh2p,
            tc.tile_pool(name="ohT", bufs=2) as ohTp,
            tc.tile_pool(name="tmp", bufs=4) as tmpp,
            tc.tile_pool(name="dram", bufs=1, space="DRAM") as dram,
        ):
            # ---- constants ----
            ident = consts.tile([P, P], f32)
            nc.sync.dma_start(out=ident, in_=ident_c[:])
            ident_bf = consts.tile([P, P], bf16)
            nc.scalar.copy(ident_bf, ident)
            # iota_row[p, d] = d  (pass-A one-hot compare operand)
            iota_i32 = consts.tile([P, D], i32)
            nc.gpsimd.iota(iota_i32, pattern=[[1, D]], base=0, channel_multiplier=0)
            iota_row = consts.tile([P, D], f32)
            nc.vector.tensor_copy(out=iota_row, in_=iota_i32)
            # iota32[p, t*32 + rb*16 + r*8 + d] = d + 8*rb: rb=0 rows build
            # the doubled one-hot, rb=1 rows (values 8..15) never match y and
            # pad each tile's K to 32 so lhsT slices start at 0/32/64/96
            iota32_i32 = consts.tile([P, CHUNK * 4 * D], i32)
            nc.gpsimd.iota(
                iota32_i32, pattern=[[0, CHUNK], [D, 2], [0, 2], [1, D]],
                base=0, channel_multiplier=0,
            )
            iota32 = consts.tile([P, CHUNK * 4 * D], f32)
            nc.vector.tensor_copy(out=iota32, in_=iota32_i32)
            gam = consts.tile([D, F], f32)
            nc.sync.dma_start(out=gam, in_=gamma[:])
            bet = consts.tile([D, F], f32)
            nc.sync.dma_start(out=bet, in_=beta[:])
            ones_bf = consts.tile([P, 1], bf16)
            nc.vector.memset(ones_bf, 1.0)
            y_cols = consts.tile([P, T], f32)
            nc.sync.dma_start(out=y_cols, in_=y_r)

            W = 2 * F + 1
            pack = tables.tile([D, W], f32)
            # transposed padded one-hots for every pass-B tile, built during
            # pass A: [l-strip partitions, chunk, quad, 128]
            ohT32_all = consts.tile([P, NCHUNKS, CHUNK // 4, P], bf16)

            # ---- pass A: segmented stats + pass-B one-hot transposes ----
            with (
                tc.tile_pool(name="stats_ps", bufs=1, space="PSUM") as stats_ps,
                tc.tile_pool(name="pT", bufs=2, space="PSUM") as pTp,
            ):
                psum_sums = stats_ps.tile([D, F], f32)
                psum_sumsq = stats_ps.tile([D, F], f32)
                psum_cnt = stats_ps.tile([D, 1], f32)

                for c in range(NCHUNKS):
                    xc = xcp.tile([P, CHUNK, F], f32)
                    nc.sync.dma_start(
                        out=xc, in_=x_r[:, c * CHUNK : (c + 1) * CHUNK, :]
                    )
                    # padded doubled one-hot + transpose for this chunk's
                    # pass-B coefficient gathers
                    ohs2 = oh2p.tile([P, CHUNK * 4 * D], bf16)
                    ysl = y_cols[:, c * CHUNK : (c + 1) * CHUNK]
                    ybc = bass.AP(
                        tensor=ysl.tensor,
                        offset=ysl.offset,
                        ap=list(ysl.ap) + [[0, 4 * D]],
                    )
                    nc.vector.tensor_tensor(
                        ohs2.rearrange("p (k r) -> p k r", r=4 * D),
                        iota32.rearrange("p (k r) -> p k r", r=4 * D),
                        ybc,
                        AluOp.is_equal,
                    )
                    for h in range(CHUNK // 4):
                        psum_oT = pTp.tile([P, P], f32)
                        nc.tensor.matmul(
                            psum_oT,
                            ohs2[:, h * P : (h + 1) * P],
                            ident_bf,
                            start=True, stop=True, skip_group_check=True,
                        )
                        nc.scalar.copy(ohT32_all[:, c, h, :], psum_oT)
                    for k in range(CHUNK):
                        t = c * CHUNK + k
                        oh = ohp.tile([P, D], bf16)
                        nc.vector.tensor_tensor(
                            oh, iota_row,
                            y_cols[:, t : t + 1].to_broadcast([P, D]),
                            AluOp.is_equal,
                        )
                        xb = xbp.tile([P, F], bf16)
                        nc.vector.tensor_copy(out=xb, in_=xc[:, k, :])
                        xsq = xsqp.tile([P, F], bf16)
                        nc.scalar.square(xsq, xc[:, k, :])
                        first = t == 0
                        last = t == T - 1
                        nc.tensor.matmul(
                            psum_sums, oh, xb,
                            start=first, stop=last, skip_group_check=True,
                        )
                        nc.tensor.matmul(
                            psum_sumsq, oh, xsq,
                            start=first, stop=last, skip_group_check=True,
                        )
                        nc.tensor.matmul(
                            psum_cnt, oh, ones_bf,
                            start=first, stop=last, skip_group_check=True,
                        )

                # ---- pack stats ----
                nc.scalar.copy(pack[:, 0:F], psum_sums)
                nc.scalar.copy(pack[:, F : 2 * F], psum_sumsq)
                nc.scalar.copy(pack[:, 2 * F : W], psum_cnt)

            # ---- allreduce ----
            cc_in = dram.tile([D, W], f32)
            cc_out = dram.tile([D, W], f32)
            nc.gpsimd.dma_start(out=cc_in, in_=pack)
            nc.gpsimd.collective_compute(
                "AllReduce",
                AluOp.add,
                replica_groups=[list(range(N_CORES))],
                ins=[cc_in.opt()],
                outs=[cc_out.opt()],
            )
            red = tables.tile([D, W], f32, tag="pack")
            nc.gpsimd.dma_start(out=red, in_=cc_out)
            S = red[:, 0:F]
            Q = red[:, F : 2 * F]
            cnt = red[:, 2 * F : W]

            # ---- table math (all [8, F] / [8, 1]) ----
            safe = tables.tile([D, 1], f32)
            nc.vector.tensor_scalar(safe, cnt, 1.0, None, AluOp.max)
            rn = tables.tile([D, 1], f32)
            nc.vector.reciprocal(rn, safe)
            mean = tables.tile([D, F], f32)
            nc.vector.tensor_scalar(mean, S, rn, None, AluOp.mult)
            ex2 = tables.tile([D, F], f32)
            nc.vector.tensor_scalar(ex2, Q, rn, None, AluOp.mult)
            mb = tables.tile([D, 1], f32)
            nc.vector.tensor_scalar(mb, cnt, 1.0, None, AluOp.is_gt)
            omb = tables.tile([D, 1], f32)
            nc.vector.tensor_scalar(omb, mb, -1.0, 1.0, AluOp.mult, AluOp.add)
            nz = tables.tile([D, 1], f32)
            nc.vector.tensor_scalar(nz, cnt, 0.0, None, AluOp.is_gt)
            mean_e = tables.tile([D, F], f32)
            nc.vector.tensor_scalar(mean_e, mean, mb, None, AluOp.mult)
            m2 = tables.tile([D, F], f32)
            nc.vector.tensor_tensor(m2, mean, mean, AluOp.mult)
            var = tables.tile([D, F], f32)
            nc.vector.tensor_tensor(var, ex2, m2, AluOp.subtract)
            var_e = tables.tile([D, F], f32)
            nc.vector.tensor_scalar(var_e, var, mb, omb, AluOp.mult, AluOp.add)
            eps_t = tables.tile([D, 1], f32)
            nc.vector.memset(eps_t, EPS)
            sd = tables.tile([D, F], f32)
            nc.scalar.activation(
                sd, var_e, mybir.ActivationFunctionType.Sqrt, bias=eps_t[:, 0:1]
            )
            inv = tables.tile([D, F], f32)
            nc.vector.reciprocal(inv, sd)
            A = tables.tile([D, F], f32)
            nc.vector.scalar_tensor_tensor(A, gam, nz, inv, AluOp.mult, AluOp.mult)
            t1 = tables.tile([D, F], f32)
            nc.vector.tensor_tensor(t1, A, mean_e, AluOp.mult)
            B = tables.tile([D, F], f32)
            nc.vector.scalar_tensor_tensor(
                B, bet, nz, t1, AluOp.mult, AluOp.subtract
            )

            # ---- hi/lo bf16 split of A and B, stacked along K ----
            # AHL[0:8] = bf16(A); AHL[8:16] = bf16(A - f32(bf16(A)))
            # (engines can only address partition windows starting at 0/32/64,
            # so the lo halves go through a tiny SBUF->SBUF DMA)
            # replicated to partition bases 0/32/64/96 because the PE
            # requires lhsT and rhs to share a base partition
            AHL = tables.tile([P, F], bf16)
            BHL = tables.tile([P, F], bf16)
            nc.vector.memset(AHL, 0.0)
            nc.vector.memset(BHL, 0.0)
            hi32 = tables.tile([D, F], f32)
            res = tables.tile([D, F], f32)
            lo_bf = tables.tile([D, F], bf16)
            nc.scalar.copy(AHL[0:D, :], A)
            nc.scalar.copy(hi32, AHL[0:D, :])
            nc.vector.tensor_tensor(res, A, hi32, AluOp.subtract)
            nc.scalar.copy(lo_bf, res)
            nc.sync.dma_start(out=AHL[D : 2 * D, :], in_=lo_bf)
            for l in range(1, 4):
                nc.sync.dma_start(
                    out=AHL[l * 32 : l * 32 + 2 * D, :], in_=AHL[0 : 2 * D, :]
                )
            hi32b = tables.tile([D, F], f32)
            resb = tables.tile([D, F], f32)
            lo_bfb = tables.tile([D, F], bf16)
            nc.scalar.copy(BHL[0:D, :], B)
            nc.scalar.copy(hi32b, BHL[0:D, :])
            nc.vector.tensor_tensor(resb, B, hi32b, AluOp.subtract)
            nc.scalar.copy(lo_bfb, resb)
            nc.sync.dma_start(out=BHL[D : 2 * D, :], in_=lo_bfb)
            for l in range(1, 4):
                nc.sync.dma_start(
                    out=BHL[l * 32 : l * 32 + 2 * D, :], in_=BHL[0 : 2 * D, :]
                )

            # ---- pass B: normalize ----
            with (
                tc.tile_pool(name="pA", bufs=2, space="PSUM") as pAp,
                tc.tile_pool(name="pB", bufs=2, space="PSUM") as pBp,
            ):
                for c in range(NCHUNKS):
                    xc = xcp.tile([P, CHUNK, F], f32)
                    nc.sync.dma_start(
                        out=xc, in_=x_r[:, c * CHUNK : (c + 1) * CHUNK, :]
                    )
                    oc = ocp.tile([P, CHUNK, F], f32)
                    for j in range(CHUNK // 2):
                        pA2 = pAp.tile([P, 2, F], f32)
                        pB2 = pBp.tile([P, 2, F], f32)
                        for i in range(2):
                            k = 2 * j + i
                            h, l = divmod(k, 4)
                            lhs = ohT32_all[l * 32 : (l + 1) * 32, c, h, :]
                            rhsA = AHL[l * 32 : (l + 1) * 32, :]
                            rhsB = BHL[l * 32 : (l + 1) * 32, :]
                            nc.tensor.matmul(
                                pA2[:, i, :], lhs, rhsA, start=True, stop=True,
                                skip_group_check=True,
                                tile_position=(l * 32, 0),
                            )
                            nc.tensor.matmul(
                                pB2[:, i, :], lhs, rhsB, start=True, stop=True,
                                skip_group_check=True,
                                tile_position=(l * 32, 0),
                            )
                        tmp2 = tmpp.tile([P, 2, F], f32)
                        nc.vector.tensor_tensor(
                            tmp2, xc[:, 2 * j : 2 * j + 2, :], pA2, AluOp.mult
                        )
                        nc.vector.tensor_tensor(
                            oc[:, 2 * j : 2 * j + 2, :], tmp2, pB2, AluOp.add
                        )
                    nc.gpsimd.dma_start(
                        out=out_r[:, c * CHUNK : (c + 1) * CHUNK, :], in_=oc
                    )

    nc.finalize()
    return nc


def _get_nc():
    if "nc" not in _CACHE:
        _CACHE["nc"] = _build()
    return _CACHE["nc"]


def kernel(x, y, gamma, beta):
    global LAST_RESULTS
    x = np.ascontiguousarray(np.asarray(x), dtype=np.float32)
    yf = np.ascontiguousarray(np.asarray(y).astype(np.float32))
    gamma = np.ascontiguousarray(np.asarray(gamma), dtype=np.float32)
    beta = np.ascontiguousarray(np.asarray(beta), dtype=np.float32)

    nc = _get_nc()
    in_maps = [
        {
            "x": x[i * NS : (i + 1) * NS],
            "yf": yf[i * NS : (i + 1) * NS],
            "gamma": gamma,
            "beta": beta,
        }
        for i in range(N_CORES)
    ]
    res = run_bass_kernel_spmd(nc, in_maps, core_ids=list(range(N_CORES)), trace=TRACE)
    LAST_RESULTS = res
    return np.concatenate([res.results[i]["out"] for i in range(N_CORES)], axis=0)
